# revision 1
# baseline (speedup 1.0000x reference)
"""Trainium2 Bass kernel for nn_LossMatch: loss = 80 * mean(|e[b,k,d] - W[d, i[b]]|).

Host side: data-parallel over B across 8 cores; the host gathers the 32
needed columns of W per core (per the sharding hint) and ships e as
fp8_e4m3 (values |x|<240 so OCP == TRN encodings) plus the per-core
replicated target, also fp8. SWDGE cast-DMAs widen both to bf16 on the way
into SBUF, halving HBM reads while keeping DVE in its fast 2x mode; a
couple of mid-stream tiles stay fp8 in SBUF (DVE 1x) to shave DMA further.

Device kernel, per [128, 2048] tile (modes in TILE_MODES):

  A: DVE tensor_tensor(sub) -> diff; ACT Abs(diff) with accum_out -> a
     partials column. Consecutive A tiles share one [128, 2*D] diff buffer
     so ACT runs one wide pass for two tiles (halves ACT op overhead).
  V: DVE tensor_tensor(max e,trep) -> mx; PE ones-matmuls accumulate
     sum(e) into PSUM bank PS_E and sum(mx) into PS_MX. With
     |e-t| = 2*max(e,t) - e - t, the host combines 2*PS_MX - PS_E
     - nV*PS_TREP (PS_TREP = ones-matmul sums of trep, done once).
     A single `ones` weights vector serves every matmul, so the PE loads
     weights once and streams matmuls back-to-back.
  R: DVE sub + DVE tensor_reduce(add, abs) -> partials column (all-DVE).

The final tile is fetched in two half-DMAs and processed as two half
tiles (max halves feeding hot-PE matmuls) to minimize the post-stream
tail. partials is written out as soon as the last ACT accumulation is
done; PSUM evacuations ride the ACT engine.
"""

import numpy as np
import ml_dtypes

B, K, D = 256, 32, 2048
NCORES = 8
BPC = B // NCORES            # 32
ROWS = BPC * K               # 1024
NTILES = ROWS // 128         # 8
MATCH_WEIGHT = 80.0

TILE_MODES = "AAAAVVVV"
E8_TILES = (2, 3, 4)         # tiles kept fp8 in SBUF (no cast; DVE eats 1x)
EBUFS = 6
NMM = 512
SPLIT_LAST = True

_cached = {}


def _split_multiwaits(nc, max_waits=1):
    """This walrus build rejects instructions carrying more than one sync
    wait: split extras into same-engine NOP chains placed just before."""
    import bass_rust

    for f in nc.m.functions:
        for bb in f.blocks:
            insts = bb.instructions
            fixups = []
            for idx, ins in enumerate(insts):
                si = ins.sync_info
                waits = list(si.on_wait) if si is not None and si.on_wait else []
                if len(waits) > max_waits:
                    fixups.append((idx, ins, waits))
            for idx, ins, waits in reversed(fixups):
                carried, kept = waits[:-max_waits], waits[-max_waits:]
                ins.sync_info.on_wait = kept
                nops = []
                for wv in carried:
                    n = nc.engines[ins.engine].nop(nofuse=True)
                    n.ins.sync_info = bass_rust.SyncInfo(on_wait=[wv], on_update=[])
                    for b2 in f.blocks:
                        if n.ins in b2.instructions:
                            b2.instructions.remove(n.ins)
                    nops.append(n.ins)
                insts[idx:idx] = nops
    return nc


def _build_nc(modes=None, unroll=1, ebufs=None, split_last=None, e8=None):
    import concourse.bass as bass
    import concourse.tile as tile
    from concourse import mybir

    AL = mybir.AluOpType
    AF = mybir.ActivationFunctionType

    modes = TILE_MODES if modes is None else modes
    ebufs = EBUFS if ebufs is None else ebufs
    split_last = SPLIT_LAST if split_last is None else split_last
    e8 = set(E8_TILES if e8 is None else e8)
    e8 = {t for t in e8 if modes[t] in "AV"}

    v_tiles = [t for t in range(NTILES) if modes[t] == "V"]
    a_tiles = [t for t in range(NTILES) if modes[t] == "A"]
    r_tiles = [t for t in range(NTILES) if modes[t] == "R"]
    nV = len(v_tiles)
    use_pe = bool(v_tiles)

    # A tiles are processed in consecutive pairs sharing one diff buffer so
    # ACT can do a single wide abs pass per pair.
    a_pair_of = {}
    for idx in range(0, len(a_tiles) - 1, 2):
        a_pair_of[a_tiles[idx]] = ("lead", a_tiles[idx + 1])
        a_pair_of[a_tiles[idx + 1]] = ("trail", a_tiles[idx])
    if len(a_tiles) % 2:
        a_pair_of[a_tiles[-1]] = ("solo", None)

    # partials columns: one per ACT pass (pair or solo) + one per R tile
    cols = []
    for t in a_tiles:
        kind, _ = a_pair_of[t]
        if kind in ("lead", "solo"):
            cols.append(("a", t))
    for t in r_tiles:
        cols.append(("r", t))
    p_cols = max(len(cols), 1)
    col_of = {k: i for i, k in enumerate(cols)}

    nc = bass.Bass()
    e = nc.dram_tensor("e", [ROWS, D], mybir.dt.float8e4, kind="ExternalInput")
    trep = nc.dram_tensor("trep", [128, D], mybir.dt.bfloat16,
                          kind="ExternalInput")
    out = nc.dram_tensor("partials", [128, p_cols], mybir.dt.float32,
                         kind="ExternalOutput")
    if use_pe:
        # pe_out columns: [sum_e | sum_mx | sum_trep]
        pe_out = nc.dram_tensor("pe_out", [1, 3 * NMM], mybir.dt.float32,
                                kind="ExternalOutput")

    with tile.TileContext(nc) as tc:
        with (
            tc.tile_pool(name="singles", bufs=1) as singles,
            tc.tile_pool(name="epool", bufs=ebufs) as epool,
            tc.tile_pool(name="dpool", bufs=3) as dpool,
            tc.tile_pool(name="mpool", bufs=4) as mpool,
            tc.tile_pool(name="apool", bufs=2) as apool,
            tc.tile_pool(name="pspool", bufs=1, space="PSUM") as pspool,
        ):
            trep_t = singles.tile([128, D], mybir.dt.bfloat16, name="trep_t")
            nc.sync.dma_start(out=trep_t[:], in_=trep[:])
            partials = singles.tile([128, p_cols], mybir.dt.float32,
                                    name="partials_t")
            if not cols:
                nc.gpsimd.memset(partials[:], 0.0)

            if use_pe:
                ones = singles.tile([128, 1], mybir.dt.bfloat16, name="ones")
                nc.gpsimd.memset(ones[:], 1.0)
                ps_e = pspool.tile([1, NMM], mybir.dt.float32, name="ps_e")
                ps_mx = pspool.tile([1, NMM], mybir.dt.float32, name="ps_mx")
                ps_tr = pspool.tile([1, NMM], mybir.dt.float32, name="ps_tr")
                evac = singles.tile([1, 3 * NMM], mybir.dt.float32,
                                    name="evac")
                banks = {"e": ps_e, "mx": ps_mx, "tr": ps_tr}
                totals = {
                    "e": 4 * nV * unroll,
                    "mx": 4 * nV * unroll,
                    "tr": 4 * unroll,
                }
                state = {k: [False, 0] for k in banks}  # started, done

                def mm(bank, src):
                    st = state[bank]
                    first = not st[0]
                    st[0] = True
                    st[1] += 1
                    nc.tensor.matmul(banks[bank][:], ones[:], src,
                                     start=first,
                                     stop=(st[1] == totals[bank]))

            pair_bufs = {}

            def do_tile(t, et, halves=1):
                mode = modes[t]
                hwd = D // halves
                if mode == "V":
                    for j in range(D // NMM):
                        mm("e", et[:, j * NMM:(j + 1) * NMM])
                    for h in range(halves):
                        hs = slice(h * hwd, (h + 1) * hwd)
                        mx = mpool.tile([128, hwd], mybir.dt.bfloat16,
                                        name=f"mx{halves}{t}", tag=f"mx{halves}")
                        nc.vector.tensor_tensor(out=mx[:], in0=et[:, hs],
                                                in1=trep_t[:, hs], op=AL.max)
                        for j in range(hwd // NMM):
                            mm("mx", mx[:, j * NMM:(j + 1) * NMM])
                elif mode == "R":
                    diff = dpool.tile([128, D], mybir.dt.bfloat16,
                                      name=f"diffR{t}", tag="diffR")
                    nc.vector.tensor_tensor(out=diff[:], in0=et,
                                            in1=trep_t[:], op=AL.subtract)
                    ci = col_of[("r", t)]
                    nc.vector.tensor_reduce(
                        out=partials[:, ci:ci + 1], in_=diff[:],
                        axis=mybir.AxisListType.X, op=AL.add,
                        apply_absolute_value=True)
                else:  # A
                    kind, other = a_pair_of[t]
                    if kind == "solo":
                        diff = dpool.tile([128, D], mybir.dt.bfloat16,
                                          name=f"diffS{t}", tag="diffS")
                        nc.vector.tensor_tensor(out=diff[:], in0=et,
                                                in1=trep_t[:], op=AL.subtract)
                        absd = apool.tile([128, D], mybir.dt.bfloat16,
                                          name=f"absdS{t}", tag="absdS")
                        ci = col_of[("a", t)]
                        nc.scalar.activation(
                            out=absd[:], in_=diff[:], func=AF.Abs,
                            accum_out=partials[:, ci:ci + 1])
                        return
                    if kind == "lead":
                        buf = dpool.tile([128, 2 * D], mybir.dt.bfloat16,
                                         name=f"diffP{t}", tag="diffP")
                        pair_bufs[t] = buf
                        half = 0
                        lead = t
                    else:
                        buf = pair_bufs.pop(other)
                        half = 1
                        lead = other
                    nc.vector.tensor_tensor(
                        out=buf[:, half * D:(half + 1) * D], in0=et,
                        in1=trep_t[:], op=AL.subtract)
                    if half == 1:
                        absd = apool.tile([128, 2 * D], mybir.dt.bfloat16,
                                          name=f"absdP{t}", tag="absdP")
                        ci = col_of[("a", lead)]
                        nc.scalar.activation(
                            out=absd[:], in_=buf[:], func=AF.Abs,
                            accum_out=partials[:, ci:ci + 1])

            for rep in range(unroll):
                if use_pe:
                    for j in range(D // NMM):
                        mm("tr", trep_t[:, j * NMM:(j + 1) * NMM])
                for t in range(NTILES):
                    keep8 = t in e8
                    dt = mybir.dt.float8e4 if keep8 else mybir.dt.bfloat16
                    tag = "ec8" if keep8 else "ec"
                    ec = epool.tile([128, D], dt, name=tag, tag=tag)
                    last = split_last and t == NTILES - 1
                    if last:
                        for h in range(2):
                            hs = slice(h * (D // 2), (h + 1) * (D // 2))
                            nc.gpsimd.dma_start(
                                out=ec[:, hs],
                                in_=e[t * 128:(t + 1) * 128, hs])
                    else:
                        nc.gpsimd.dma_start(
                            out=ec[:], in_=e[t * 128:(t + 1) * 128, :])
                    halves = 2 if (last and modes[t] == "V") else 1
                    do_tile(t, ec[:], halves=halves)
                    # evacuate the trep PSUM bank early, off the tail
                    if use_pe and t == 1 and rep == 0:
                        nc.scalar.copy(out=evac[:, 2 * NMM:], in_=ps_tr[:])

            nc.scalar.dma_start(out=out[:], in_=partials[:])
            if use_pe:
                nc.scalar.copy(out=evac[:, 0:NMM], in_=ps_e[:])
                nc.scalar.copy(out=evac[:, NMM:2 * NMM], in_=ps_mx[:])
                nc.sync.dma_start(out=pe_out[:], in_=evac[:])
    return _split_multiwaits(nc)


def _prepare_in_maps(e_vectors, W, i):
    e = np.asarray(e_vectors, dtype=np.float32).reshape(B, K, D)
    idx = np.asarray(i).astype(np.int64)
    target = np.ascontiguousarray(W[:, idx].T)  # [B, D]

    # Block-repeat partition layout: tile t covers k = 4t + j, row index
    # within a tile is p = b_local + 32*j  ->  global row 128*t + 32*j + b.
    e_sh = (
        e.reshape(NCORES, BPC, K // 4, 4, D)
        .transpose(0, 2, 3, 1, 4)
        .reshape(NCORES, ROWS, D)
        .astype(ml_dtypes.float8_e4m3fn)
    )
    t_sh = target.astype(ml_dtypes.bfloat16)

    in_maps = []
    for c in range(NCORES):
        in_maps.append({
            "e": np.ascontiguousarray(e_sh[c]),
            "trep": np.ascontiguousarray(
                np.tile(t_sh[c * BPC:(c + 1) * BPC], (4, 1))),
        })
    return in_maps


def _run(e_vectors, W, i, **spmd_kwargs):
    from concourse.bass_utils import run_bass_kernel_spmd

    if "nc" not in _cached:
        _cached["nc"] = _build_nc()
    in_maps = _prepare_in_maps(e_vectors, W, i)
    res = run_bass_kernel_spmd(_cached["nc"], in_maps,
                               core_ids=list(range(NCORES)), **spmd_kwargs)
    nV = TILE_MODES.count("V")
    total = 0.0
    for r in res.results:
        total += np.asarray(r["partials"], dtype=np.float64).sum()
        if "pe_out" in r:
            p = np.asarray(r["pe_out"], dtype=np.float64).reshape(3, NMM)
            total += 2.0 * p[1].sum() - p[0].sum() - nV * p[2].sum()
    loss = MATCH_WEIGHT * total / float(B * K * D)
    return np.float32(loss), res


def kernel(e_vectors, W, i):
    loss, _ = _run(e_vectors, W, i)
    return loss

